# revision 5
# baseline (speedup 1.0000x reference)
"""Trainium2 Bass kernel for nn_CubicalModel_ISM.

Computes Xp = X @ p and Yp = Y @ p (X, Y: [784, 32768] f32, p: [32768] f32),
then gathers persistence-diagram values from the reshaped [28, 28] images.

Sharding: q (parameter) axis split across 8 NeuronCores, 4096 columns each.
Each core streams its [784, 4096] shards of X and Y through SBUF and does a
fused multiply + free-axis reduce on the Vector engine (scalar_tensor_tensor
with accum_out), producing per-core partial row sums [784] per tensor. The
[784] partials are summed across cores on the host (tiny), and the
200-element gathers run on the host as well.

Per-core layout: partition p holds rows 7p..7p+6 of the [784, 4096] shard
(112 partitions x 7 rows). Rows within a partition are consecutive, so any
row-slice is one contiguous DRAM run per partition — DMA chunks of 2 rows
move 32KB contiguous per partition. DMA instructions are round-robined
across the SP/ACT HWDGE rings and the GpSimd SWDGE ring so all 16 SDMA
engines stay fed through per-instruction completion bubbles.
"""

import numpy as np

H = W = 28
Q = 32768
N_CORES = 8
QS = Q // N_CORES  # 4096 per-core q shard
R = H * W          # 784 rows
P = 112            # SBUF partitions used
RPP = R // P       # 7 rows per partition

# row-chunking of the 7 rows per partition: DMA granularity
ROW_CHUNKS = [(0, 2), (2, 4), (4, 6), (6, 7)]

_CACHE = {}


def _build_nc():
    import concourse.bacc as bacc
    import concourse.mybir as mybir
    from concourse.tile import TileContext

    # Bacc (not raw Bass) is required: its compile() runs
    # generate_event_semaphores, which splits multi-wait instructions into
    # the 1-wait-per-instruction form this walrus accepts.
    nc = bacc.Bacc(None)
    f32 = mybir.dt.float32
    x = nc.dram_tensor("x", [R, QS], f32, kind="ExternalInput")
    y = nc.dram_tensor("y", [R, QS], f32, kind="ExternalInput")
    p = nc.dram_tensor("p", [1, QS], f32, kind="ExternalInput")
    out = nc.dram_tensor("out", [P, 2 * RPP], f32, kind="ExternalOutput")

    # [784, 4096] -> [112, 7*4096]: partition p's free span = rows 7p..7p+6
    xv = x[:, :].rearrange("(p r) q -> p (r q)", p=P)
    yv = y[:, :].rearrange("(p r) q -> p (r q)", p=P)

    rings = ["sync", "scalar", "gpsimd"]

    with TileContext(nc) as tc:
        with (
            tc.tile_pool(name="pbpool", bufs=1) as pb_pool,
            tc.tile_pool(name="chunks", bufs=4) as chunk_pool,
            tc.tile_pool(name="scratch", bufs=1) as scratch_pool,
            tc.tile_pool(name="respool", bufs=1) as res_pool,
        ):
            p_row = pb_pool.tile([1, QS], f32)
            pb = pb_pool.tile([P, QS], f32)
            nc.sync.dma_start(out=p_row[:, :], in_=p[:, :])
            nc.gpsimd.partition_broadcast(pb[:, :], p_row[:, :], channels=P)

            res = res_pool.tile([P, 2 * RPP], f32)
            scratch = scratch_pool.tile([P, QS], f32)
            ring_i = 0
            for t, src in enumerate((xv, yv)):
                for r0, r1 in ROW_CHUNKS:
                    nrows = r1 - r0
                    chunk = chunk_pool.tile([P, nrows * QS], f32, tag="chunk")
                    eng = getattr(nc, rings[ring_i % len(rings)])
                    ring_i += 1
                    eng.dma_start(
                        out=chunk[:, :], in_=src[:, r0 * QS : r1 * QS]
                    )
                    for j in range(nrows):
                        col = t * RPP + r0 + j
                        # out = (chunk_row * 1.0) * pb elementwise (into
                        # scratch, discarded); accum_out = per-partition sum
                        # — fused multiply + reduce in one DVE pass.
                        nc.vector.scalar_tensor_tensor(
                            out=scratch[:, :],
                            in0=chunk[:, j * QS : (j + 1) * QS],
                            scalar=1.0,
                            in1=pb[:, :],
                            op0=mybir.AluOpType.mult,
                            op1=mybir.AluOpType.mult,
                            accum_out=res[:, col : col + 1],
                        )
            nc.sync.dma_start(out=out[:, :], in_=res[:, :])
    nc.finalize()
    return nc


def _get_nc():
    if "nc" not in _CACHE:
        _CACHE["nc"] = _build_nc()
    return _CACHE["nc"]


def _make_in_maps(X, Y, p):
    in_maps = []
    for c in range(N_CORES):
        sl = slice(c * QS, (c + 1) * QS)
        in_maps.append(
            {
                "x": np.ascontiguousarray(X[:, sl]),
                "y": np.ascontiguousarray(Y[:, sl]),
                "p": np.ascontiguousarray(p[sl]).reshape(1, QS),
            }
        )
    return in_maps


def kernel(X, Y, p, inds1, inds2):
    from concourse.bass_utils import run_bass_kernel_spmd

    X = np.asarray(X, dtype=np.float32)
    Y = np.asarray(Y, dtype=np.float32)
    p = np.asarray(p, dtype=np.float32)
    inds1 = np.asarray(inds1)
    inds2 = np.asarray(inds2)

    nc = _get_nc()
    results = run_bass_kernel_spmd(
        nc, _make_in_maps(X, Y, p), list(range(N_CORES))
    ).results

    xp = np.zeros(R, dtype=np.float32)
    yp = np.zeros(R, dtype=np.float32)
    for c in range(N_CORES):
        o = results[c]["out"]  # [112, 14]; [p, k] = row 7p + (k mod 7)
        xp += o[:, :RPP].reshape(R)
        yp += o[:, RPP:].reshape(R)

    def gather(img, inds):
        ij = inds.reshape(-1, 2)
        return img[ij[:, 0], ij[:, 1]].reshape(-1, 2)

    dgm1 = gather(xp.reshape(H, W), inds1)
    dgm2 = gather(yp.reshape(H, W), inds2)
    return dgm1, dgm2
